# revision 7
# baseline (speedup 1.0000x reference)
"""Causal multi-head attention (B=2, T=2048, E=1024, 16 heads) on 8 TRN2 cores.

Sharding: 8-way tensor-parallel over heads (2 heads/core) for QKV projections
and attention; one AllToAll per head-half re-shards the attention output over
tokens so each core computes the output projection for its 512-token block.

v3 (vs v2 baseline, 218.9us):
- both heads of a chunk processed together; the two heads' score matmuls
  (64-row contraction each) are emitted back-to-back so tile_position
  row-groups (0,0)/(64,0) run them CONCURRENTLY in the PE array -> scores
  PE time halves.
- xP host layout [p, t, e, n]: chunk DMAs are 8KB-contiguous per partition
  (was 1KB packets via the strided xT rearrange).
- 32 warmup matmuls on the eye tile during the initial DMA wait: HAM flips
  to K=8/8 before the first real QKV matmul instead of ~4us into it.
- tail: A2A#0 fires right after chunk-7 h0 normalize; h1 normalize +
  wo loads + proj-h0 overlap A2A#1.
- y stored bf16 (halves the final DMA; host casts back to f32).
"""
import sys

if "/opt/trn_rl_repo" not in sys.path:
    sys.path.insert(0, "/opt/trn_rl_repo")

import numpy as np

import concourse.bacc as bacc
import concourse.mybir as mybir
from concourse import tile
from concourse.bass_utils import run_bass_kernel_spmd

dt = mybir.dt
AF = mybir.ActivationFunctionType
ALU = mybir.AluOpType

B, T, E, HS, NH = 2, 2048, 1024, 64, 16
NCORE = 8
NTOK = B * T            # 4096
CH = 512                # token chunk
NCH = NTOK // CH        # 8
CPB = NCH // B          # chunks per batch = 4
SUB = 128
NSUB = CH // SUB        # 4

_nc_cache = {}


def build_nc():
    nc = bacc.Bacc("TRN2", target_bir_lowering=False, debug=False,
                   num_devices=NCORE)
    f32, bf16 = dt.float32, dt.bfloat16

    xP = nc.declare_dram_parameter("xP", [128, NCH, 8, CH], bf16,
                                   isOutput=False)
    wqT = nc.declare_dram_parameter("wqT", [128, 8, 128], bf16,
                                    isOutput=False)
    wkT = nc.declare_dram_parameter("wkT", [128, 8, 128], bf16,
                                    isOutput=False)
    wvT = nc.declare_dram_parameter("wvT", [128, 8, 128], bf16,
                                    isOutput=False)
    woh0 = nc.declare_dram_parameter("woh0", [512, E], bf16, isOutput=False)
    woh1 = nc.declare_dram_parameter("woh1", [512, E], bf16, isOutput=False)
    bqs = nc.declare_dram_parameter("bqs", [128, 1], f32, isOutput=False)
    bks = nc.declare_dram_parameter("bks", [128, 1], f32, isOutput=False)
    bvs = nc.declare_dram_parameter("bvs", [128, 1], f32, isOutput=False)
    bo_b = nc.declare_dram_parameter("bo_b", [128, E], f32, isOutput=False)
    eye = nc.declare_dram_parameter("eye", [128, 128], bf16, isOutput=False)
    tri01 = nc.declare_dram_parameter("tri01", [128, 128], bf16,
                                      isOutput=False)
    ones_v = nc.declare_dram_parameter("ones_v", [128, NCH * NSUB], bf16,
                                       isOutput=False)
    y = nc.declare_dram_parameter("y", [CH, E], bf16, isOutput=True)

    with tile.TileContext(nc) as tc:
        from contextlib import ExitStack
        with ExitStack() as top:
            const = top.enter_context(tc.tile_pool(name="const", bufs=1))
            persist = top.enter_context(tc.tile_pool(name="persist", bufs=1))
            xtp_pool = top.enter_context(tc.tile_pool(name="xtp", bufs=2))
            vstage = top.enter_context(tc.tile_pool(name="vstage", bufs=2))
            ppool = top.enter_context(tc.tile_pool(name="ppool", bufs=2))
            apool = top.enter_context(tc.tile_pool(name="apool", bufs=2))
            bcpool = top.enter_context(tc.tile_pool(name="bcpool", bufs=2))
            recpool = top.enter_context(tc.tile_pool(name="recpool", bufs=2))
            ystage = top.enter_context(tc.tile_pool(name="ystage", bufs=2))
            ps_q = top.enter_context(
                tc.tile_pool(name="ps_q", bufs=2, space="PSUM"))
            ps_s0 = top.enter_context(
                tc.tile_pool(name="ps_s0", bufs=2, space="PSUM"))
            ps_s1 = top.enter_context(
                tc.tile_pool(name="ps_s1", bufs=2, space="PSUM"))
            ps_a0 = top.enter_context(
                tc.tile_pool(name="ps_a0", bufs=1, space="PSUM"))
            ps_a1 = top.enter_context(
                tc.tile_pool(name="ps_a1", bufs=1, space="PSUM"))
            dram = top.enter_context(
                tc.tile_pool(name="dram", bufs=1, space="DRAM"))

            # ---- eye first: unblocks the HAM warmup matmuls ---------------
            eye_sb = const.tile([128, 128], bf16, name="eye_sb")
            nc.sync.dma_start(eye_sb[:], eye[:])

            # ---- HAM warmup: PE busy during the initial DMA wait ----------
            wps = ps_q.tile([128, 128], f32, name="wps", tag="psq")
            for _ in range(36):
                nc.tensor.matmul(wps[:], eye_sb[:], eye_sb[:],
                                 start=True, stop=True)

            # ---- persistent weights (emitted early: unblock chunk 0) ------
            wq_sb = persist.tile([128, 8, 128], bf16, name="wq_sb")
            wk_sb = persist.tile([128, 8, 128], bf16, name="wk_sb")
            wv_sb = persist.tile([128, 8, 128], bf16, name="wv_sb")
            nc.sync.dma_start(wq_sb[:], wqT[:])
            nc.sync.dma_start(wk_sb[:], wkT[:])
            nc.sync.dma_start(wv_sb[:], wvT[:])

            # ---- remaining constants --------------------------------------
            onesv_sb = const.tile([128, NCH * NSUB], bf16, name="onesv_sb")
            nc.sync.dma_start(onesv_sb[:], ones_v[:])
            bq_sb = const.tile([128, 1], f32, name="bq_sb")
            nc.sync.dma_start(bq_sb[:], bqs[:])
            bk_sb = const.tile([128, 1], f32, name="bk_sb")
            nc.sync.dma_start(bk_sb[:], bks[:])
            bv_sb = const.tile([128, 1], f32, name="bv_sb")
            nc.sync.dma_start(bv_sb[:], bvs[:])
            tri_sb = const.tile([128, 128], bf16, name="tri_sb")
            nc.sync.dma_start(tri_sb[:], tri01[:])
            bo_sb = const.tile([128, E], f32, name="bo_sb")
            nc.sync.dma_start(bo_sb[:], bo_b[:])

            # ---- persistent activations -----------------------------------
            kT = persist.tile([128, NCH, CH], bf16, name="kT")
            qT = persist.tile([128, NCH, CH], bf16, name="qT")
            # V rows per k-token group g; cols 0:64 = h0 feats, 64 = ones,
            # 65:129 = h1 feats, 129 = ones.  AV stationary h = [:, g,
            # 65h:65h+65]; the ones row makes the AV matmul emit softmax
            # denominators in PSUM row 64.
            vh = persist.tile([128, NCH * NSUB, 130], bf16, name="vh")
            nc.vector.tensor_copy(vh[:, :, 64], onesv_sb[:])
            nc.vector.tensor_copy(vh[:, :, 129], onesv_sb[:])

            wo0_sb = persist.tile([128, 4, E], bf16, name="wo0_sb")
            wo1_sb = persist.tile([128, 4, E], bf16, name="wo1_sb")

            cc_in = [dram.tile([NCH, 64, CH], bf16, name=f"cc_in{h}")
                     for h in range(2)]
            cc_out = [dram.tile([NCH, 64, CH], bf16, name=f"cc_out{h}")
                      for h in range(2)]
            warm_in = dram.tile([NCH, 1, 32], bf16, name="warm_in")
            warm_out = dram.tile([NCH, 1, 32], bf16, name="warm_out")
            nc.sync.dma_start(warm_in[:, 0, :], onesv_sb[0:8, 0:32])

            # ---- phase B: QKV projection for one token chunk ---------------
            def emit_b(t):
                xTt = xtp_pool.tile([128, 8, CH], bf16, name="xTt", tag="xTt")
                if t == 0:
                    # per-e-tile DMAs: the first projection matmul starts
                    # after 128KB instead of the full 1MB chunk
                    for e in range(8):
                        nc.sync.dma_start(xTt[:, e, :], xP[:, 0, e, :])
                else:
                    # single DMA, 8KB contiguous per partition
                    nc.sync.dma_start(xTt[:], xP[:, t])
                for wsb, bias, scale, dest in (
                        (wq_sb, bq_sb, 0.125, qT),
                        (wk_sb, bk_sb, None, kT)):
                    ps = ps_q.tile([128, CH], f32, name="psqk", tag="psq")
                    for e in range(8):
                        nc.tensor.matmul(ps[:], wsb[:, e, :], xTt[:, e, :],
                                         start=(e == 0), stop=(e == 7))
                    if scale is None:
                        nc.vector.tensor_scalar_add(dest[:, t, :], ps[:],
                                                    bias[:])
                    else:
                        nc.vector.tensor_scalar(
                            dest[:, t, :], ps[:], scale, bias[:],
                            ALU.mult, ALU.add)

                psv = ps_q.tile([128, CH], f32, name="psv", tag="psq")
                for e in range(8):
                    nc.tensor.matmul(psv[:], wv_sb[:, e, :], xTt[:, e, :],
                                     start=(e == 0), stop=(e == 7))
                vTs = vstage.tile([128, CH], bf16, name="vTs", tag="vTs")
                nc.vector.tensor_scalar_add(vTs[:], psv[:], bv_sb[:])
                for s in range(NSUB):
                    tv = ps_q.tile([128, 128], bf16, name="tv", tag="psq")
                    nc.tensor.transpose(
                        tv[:], vTs[:, 128 * s:128 * (s + 1)], eye_sb[:])
                    g = NSUB * t + s
                    nc.vector.tensor_copy(vh[:, g, 0:64], tv[:, 0:64])
                    nc.vector.tensor_copy(vh[:, g, 65:129], tv[:, 64:128])

            # ---- phase C: attention for one chunk ---------------------------
            # heads=(0,1): both heads interleaved so the two 64-row score
            # matmuls land adjacent in the PE stream (row-groups (0,0)/(64,0)
            # overlap). heads=(h,): single head, v2-style (used for the last
            # two chunks so h1 lags and overlaps the A2As).
            def emit_c2(t, heads=(0, 1)):
                b0 = CPB * (t // CPB)
                apools = {0: ps_a0, 1: ps_a1}
                sppools = {0: ps_s0, 1: ps_s1}
                a_ps = {h: apools[h].tile([128, CH], f32, name=f"aps{h}",
                                          tag=f"aps{h}") for h in heads}

                def emit_scores(kc):
                    diag = kc == t
                    pT = {h: ppool.tile([128, NSUB, CH], bf16, name=f"pT{h}",
                                        tag=f"pT{h}") for h in heads}
                    sps = []
                    for s in range(NSUB):
                        q0 = SUB * s if diag else 0
                        pair = {}
                        for h in heads:
                            pb = 64 * h
                            sp = sppools[h].tile([128, CH], f32,
                                                 name=f"sp{h}",
                                                 tag=f"sps{h}")
                            nc.tensor.matmul(
                                sp[:, q0:CH],
                                kT[pb:pb + 64, kc, SUB * s:SUB * (s + 1)],
                                qT[pb:pb + 64, t, q0:CH],
                                start=True, stop=True)
                            pair[h] = sp
                        sps.append((pair, q0))
                    for s, (pair, q0) in enumerate(sps):
                        for h in heads:
                            nc.scalar.activation(pT[h][:, s, q0:CH],
                                                 pair[h][:, q0:CH], AF.Exp)
                        if diag:
                            for h in heads:
                                nc.vector.tensor_mul(
                                    pT[h][:, s, q0:q0 + SUB],
                                    pT[h][:, s, q0:q0 + SUB], tri_sb[:])
                    return pT

                def emit_av(kc, pT):
                    diag = kc == t
                    for s in range(NSUB):
                        q0 = SUB * s if diag else 0
                        g = NSUB * kc + s
                        for h in heads:
                            nc.tensor.matmul(
                                a_ps[h][0:65, q0:CH],
                                vh[:, g, 65 * h:65 * h + 65],
                                pT[h][:, s, q0:CH],
                                start=(kc == b0 and s == 0),
                                stop=(diag and s == NSUB - 1))

                prev = None
                for kc in range(b0, t + 1):
                    pT = emit_scores(kc)
                    if prev is not None:
                        emit_av(*prev)
                    prev = (kc, pT)
                emit_av(*prev)

                for h in heads:
                    den = recpool.tile([1, CH], f32, name="den",
                                       tag=f"den{h}")
                    nc.vector.tensor_copy(den[:], a_ps[h][64:65, :])
                    rec = recpool.tile([1, CH], f32, name="rec",
                                       tag=f"rec{h}")
                    nc.vector.reciprocal_approx_fast(out=rec[:], in_=den[:])
                    bc = bcpool.tile([64, CH], f32, name="bc", tag=f"bc{h}")
                    nc.gpsimd.partition_broadcast(bc[:], rec[:])
                    a_sb = apool.tile([64, CH], bf16, name="a_sb",
                                      tag=f"asb{h}")
                    nc.vector.tensor_mul(a_sb[:], a_ps[h][0:64, :], bc[:])
                    nc.sync.dma_start(cc_in[h][t, :, :], a_sb[:])

            # ---- main pipeline ---------------------------------------------
            # chunks 0..5: both heads merged (score row-packing).
            # chunks 6..7: v2-style head stagger — h0 finishes first, A2A#0
            # fires while h1's tail + proj-h0 run.
            for t in range(NCH):
                emit_b(t)
                if t in (4, 6):
                    # tiny dummy AllToAll: keeps the CC stream warm so the
                    # real A2As don't pay the cold-stream penalty
                    nc.gpsimd.collective_compute(
                        "AllToAll", ALU.bypass,
                        ins=[warm_in.opt()], outs=[warm_out.opt()],
                        replica_groups=[list(range(NCORE))])
                if 1 <= t <= NCH - 2:
                    emit_c2(t - 1)
            # wo weights: DMA-idle window once all xP chunks are in flight
            nc.sync.dma_start(wo0_sb[:],
                              woh0.rearrange("(r p) e -> p r e", p=128))
            nc.sync.dma_start(wo1_sb[:],
                              woh1.rearrange("(r p) e -> p r e", p=128))

            emit_c2(NCH - 2, heads=(0,))
            emit_c2(NCH - 1, heads=(0,))
            nc.gpsimd.collective_compute(
                "AllToAll", ALU.bypass,
                ins=[cc_in[0].opt()], outs=[cc_out[0].opt()],
                replica_groups=[list(range(NCORE))])

            aTb = xtp_pool.tile([128, 2, 4, CH], bf16, name="aTb", tag="xTt")
            nc.sync.dma_start(
                aTb[:, 0],
                cc_out[0].rearrange("(a two) f n -> (two f) a n", two=2))

            emit_c2(NCH - 2, heads=(1,))

            # ---- phase E0: h0 half of the output projection ---------------
            # (deps: A2A#0 + wo0 only — fills PE while h1 tail + A2A#1 run)
            yacc = persist.tile([128, NSUB, E], f32, name="yacc")

            def emit_y0():
                for m in range(NSUB):
                    for nchk in range(2):
                        yps = ps_q.tile([128, CH], f32, name="yps",
                                        tag="psq")
                        for p in range(4):
                            nc.tensor.matmul(
                                yps[:],
                                aTb[:, 0, p, SUB * m:SUB * (m + 1)],
                                wo0_sb[:, p, CH * nchk:CH * (nchk + 1)],
                                start=(p == 0), stop=(p == 3))
                        nc.vector.tensor_add(
                            yacc[:, m, CH * nchk:CH * (nchk + 1)], yps[:],
                            bo_sb[:, CH * nchk:CH * (nchk + 1)])

            emit_c2(NCH - 1, heads=(1,))
            nc.gpsimd.collective_compute(
                "AllToAll", ALU.bypass,
                ins=[cc_in[1].opt()], outs=[cc_out[1].opt()],
                replica_groups=[list(range(NCORE))])
            emit_y0()
            nc.sync.dma_start(
                aTb[:, 1],
                cc_out[1].rearrange("(a two) f n -> (two f) a n", two=2))

            # ---- phase E1: h1 half + store --------------------------------
            for m in range(NSUB):
                for nchk in range(2):
                    yps = ps_q.tile([128, CH], f32, name="yps", tag="psq")
                    for p in range(4):
                        nc.tensor.matmul(
                            yps[:],
                            aTb[:, 1, p, SUB * m:SUB * (m + 1)],
                            wo1_sb[:, p, CH * nchk:CH * (nchk + 1)],
                            start=(p == 0), stop=(p == 3))
                    ysb = ystage.tile([128, CH], bf16, name="ysb", tag="ysb")
                    nc.vector.tensor_add(
                        ysb[:], yps[:],
                        yacc[:, m, CH * nchk:CH * (nchk + 1)])
                    nc.sync.dma_start(
                        y[SUB * m:SUB * (m + 1),
                          CH * nchk:CH * (nchk + 1)],
                        ysb[:])
    nc.compile()
    return nc


def _prep_in_maps(embd_q, Wq, bq, Wk, bk, Wv, bv, Wo, bo):
    import ml_dtypes
    bf16 = ml_dtypes.bfloat16
    x = embd_q.reshape(NTOK, E).astype(np.float32)
    # xP[p, t, e, n] = x[t*512+n, e*128+p]: 8KB contiguous per partition
    # per chunk
    xPm = np.ascontiguousarray(
        x.reshape(NCH, CH, 8, 128).transpose(3, 0, 2, 1).astype(bf16))
    eye = np.eye(128, dtype=bf16)
    r = np.arange(128)
    # pT is [k-part, q-col]; mask out k > q (future tokens)
    tri01 = np.ascontiguousarray(
        np.where(r[:, None] > r[None, :], 0.0, 1.0).astype(bf16))
    ones_v = np.ones((128, NCH * NSUB), dtype=bf16)
    bo_b = np.ascontiguousarray(
        np.broadcast_to(bo.astype(np.float32), (128, E)))
    woTf = Wo.astype(np.float32).T  # [feat, out]
    # pair-interleaved per-head layouts: partition q of pair p maps to
    # feat = 128*(2p) + q  (q < 64, even kt)  or  128*(2p+1) + (q-64)
    idx = np.zeros((4, 128), dtype=np.int64)
    for p in range(4):
        idx[p, :64] = 128 * (2 * p) + np.arange(64)
        idx[p, 64:] = 128 * (2 * p + 1) + np.arange(64)
    woh0 = np.ascontiguousarray(woTf[idx.reshape(-1)].astype(bf16))
    woh1 = np.ascontiguousarray(woTf[(idx + 64).reshape(-1)].astype(bf16))

    def wlayout(W, sl):
        # [E, 128] -> [p, e, m]: contiguous 2KB/partition DMA segments
        wT = W[sl].astype(np.float32).T.astype(bf16)
        return np.ascontiguousarray(
            wT.reshape(8, 128, 128).transpose(1, 0, 2))

    in_maps = []
    for c in range(NCORE):
        sl = slice(128 * c, 128 * (c + 1))
        in_maps.append({
            "xP": xPm,
            "wqT": wlayout(Wq, sl),
            "wkT": wlayout(Wk, sl),
            "wvT": wlayout(Wv, sl),
            "woh0": woh0,
            "woh1": woh1,
            "bqs": np.ascontiguousarray(
                (bq[sl] * 0.125).reshape(128, 1), dtype=np.float32),
            "bks": np.ascontiguousarray(bk[sl].reshape(128, 1),
                                        dtype=np.float32),
            "bvs": np.ascontiguousarray(bv[sl].reshape(128, 1),
                                        dtype=np.float32),
            "bo_b": bo_b,
            "eye": eye,
            "tri01": tri01,
            "ones_v": ones_v,
        })
    return in_maps


def kernel(embd_q, Wq, bq, Wk, bk, Wv, bv, Wo, bo, _trace=False):
    if "nc" not in _nc_cache:
        _nc_cache["nc"] = build_nc()
    in_maps = _prep_in_maps(np.asarray(embd_q), np.asarray(Wq), np.asarray(bq),
                            np.asarray(Wk), np.asarray(bk), np.asarray(Wv),
                            np.asarray(bv), np.asarray(Wo), np.asarray(bo))
    import os
    tc_env = os.environ.get("TRACE_CORES")
    res = run_bass_kernel_spmd(
        _nc_cache["nc"], in_maps, list(range(NCORE)), trace=_trace,
        trace_cores=(list(range(NCORE)) if tc_env else None))
    out = np.concatenate(
        [np.asarray(res.results[c]["y"]).astype(np.float32)
         for c in range(NCORE)], axis=0)
    out = out.reshape(B, T, E)
    kernel.last_results = res
    return out


# revision 8
# speedup vs baseline: 1.1784x; 1.1784x over previous
"""Causal multi-head attention (B=2, T=2048, E=1024, 16 heads) on 8 TRN2 cores.

Sharding: 8-way tensor-parallel over heads (2 heads/core) for QKV projections
and attention; one AllToAll per head-half re-shards the attention output over
tokens so each core computes the output projection for its 512-token block.

v6 (= v2 pipeline + validated fixes; v3-v5 experiments reverted):
- v2 head-staggered pipeline: h1 attention lags h0 by 2 chunks; A2A#0
  overlaps the h1 tail + output-projection loads. (Merged-head score
  row-packing measured ZERO PE win and its psum-pool pressure forced
  ps_a bufs=1, which let the warm A2A block normalize -> 25-38us PE gaps.)
- eye DMA first + 36 warmup matmuls on it: HAM flips to K=8/8 during the
  initial DMA wait (framework preamble is ~11us; warmups dovetail into
  the chunk-0 data arrival).
- xP host layout [p, t, e, n]: every chunk (incl. 0) is ONE DMA with 8KB
  contiguous per partition (v2 used 1KB packets + 8 DGE ops for chunk 0).
- y stored bf16 (halves the final DMA; host casts back to f32).
"""
import sys

if "/opt/trn_rl_repo" not in sys.path:
    sys.path.insert(0, "/opt/trn_rl_repo")

import numpy as np

import concourse.bacc as bacc
import concourse.mybir as mybir
from concourse import tile
from concourse.bass_utils import run_bass_kernel_spmd

dt = mybir.dt
AF = mybir.ActivationFunctionType
ALU = mybir.AluOpType

B, T, E, HS, NH = 2, 2048, 1024, 64, 16
NCORE = 8
NTOK = B * T            # 4096
CH = 512                # token chunk
NCH = NTOK // CH        # 8
CPB = NCH // B          # chunks per batch = 4
SUB = 128
NSUB = CH // SUB        # 4

_nc_cache = {}


def build_nc():
    nc = bacc.Bacc("TRN2", target_bir_lowering=False, debug=False,
                   num_devices=NCORE)
    f32, bf16 = dt.float32, dt.bfloat16

    xP = nc.declare_dram_parameter("xP", [128, NCH, 8, CH], bf16,
                                   isOutput=False)
    wqT = nc.declare_dram_parameter("wqT", [128, 8, 128], bf16,
                                    isOutput=False)
    wkT = nc.declare_dram_parameter("wkT", [128, 8, 128], bf16,
                                    isOutput=False)
    wvT = nc.declare_dram_parameter("wvT", [128, 8, 128], bf16,
                                    isOutput=False)
    woh0 = nc.declare_dram_parameter("woh0", [512, E], bf16, isOutput=False)
    woh1 = nc.declare_dram_parameter("woh1", [512, E], bf16, isOutput=False)
    bqs = nc.declare_dram_parameter("bqs", [128, 1], f32, isOutput=False)
    bks = nc.declare_dram_parameter("bks", [128, 1], f32, isOutput=False)
    bvs = nc.declare_dram_parameter("bvs", [128, 1], f32, isOutput=False)
    bo_b = nc.declare_dram_parameter("bo_b", [128, E], f32, isOutput=False)
    eye = nc.declare_dram_parameter("eye", [128, 128], bf16, isOutput=False)
    tri01 = nc.declare_dram_parameter("tri01", [128, 128], bf16,
                                      isOutput=False)
    ones_v = nc.declare_dram_parameter("ones_v", [128, NCH * NSUB], bf16,
                                       isOutput=False)
    y = nc.declare_dram_parameter("y", [CH, E], bf16, isOutput=True)

    with tile.TileContext(nc) as tc:
        from contextlib import ExitStack
        with ExitStack() as top:
            const = top.enter_context(tc.tile_pool(name="const", bufs=1))
            persist = top.enter_context(tc.tile_pool(name="persist", bufs=1))
            xtp_pool = top.enter_context(tc.tile_pool(name="xtp", bufs=2))
            vstage = top.enter_context(tc.tile_pool(name="vstage", bufs=2))
            ppool = top.enter_context(tc.tile_pool(name="ppool", bufs=4))
            apool = top.enter_context(tc.tile_pool(name="apool", bufs=2))
            bcpool = top.enter_context(tc.tile_pool(name="bcpool", bufs=2))
            recpool = top.enter_context(tc.tile_pool(name="recpool", bufs=2))
            ystage = top.enter_context(tc.tile_pool(name="ystage", bufs=2))
            ps_q = top.enter_context(
                tc.tile_pool(name="ps_q", bufs=2, space="PSUM"))
            ps_s = top.enter_context(
                tc.tile_pool(name="ps_s", bufs=2, space="PSUM"))
            ps_a = top.enter_context(
                tc.tile_pool(name="ps_a", bufs=2, space="PSUM"))
            dram = top.enter_context(
                tc.tile_pool(name="dram", bufs=1, space="DRAM"))

            # ---- eye first: unblocks the HAM warmup matmuls ---------------
            eye_sb = const.tile([128, 128], bf16, name="eye_sb")
            nc.sync.dma_start(eye_sb[:], eye[:])

            # ---- HAM warmup: PE busy during the initial DMA wait ----------
            wps = ps_q.tile([128, 128], f32, name="wps", tag="psq")
            for _ in range(36):
                nc.tensor.matmul(wps[:], eye_sb[:], eye_sb[:],
                                 start=True, stop=True)

            # ---- persistent weights + chunk-0 data ------------------------
            wq_sb = persist.tile([128, 8, 128], bf16, name="wq_sb")
            wk_sb = persist.tile([128, 8, 128], bf16, name="wk_sb")
            wv_sb = persist.tile([128, 8, 128], bf16, name="wv_sb")
            nc.sync.dma_start(wq_sb[:], wqT[:])
            xTt0 = xtp_pool.tile([128, 8, CH], bf16, name="xTt", tag="xTt")
            nc.sync.dma_start(xTt0[:], xP[:, 0])
            nc.sync.dma_start(wk_sb[:], wkT[:])
            nc.sync.dma_start(wv_sb[:], wvT[:])

            # ---- remaining constants --------------------------------------
            onesv_sb = const.tile([128, NCH * NSUB], bf16, name="onesv_sb")
            nc.sync.dma_start(onesv_sb[:], ones_v[:])
            bq_sb = const.tile([128, 1], f32, name="bq_sb")
            nc.sync.dma_start(bq_sb[:], bqs[:])
            bk_sb = const.tile([128, 1], f32, name="bk_sb")
            nc.sync.dma_start(bk_sb[:], bks[:])
            bv_sb = const.tile([128, 1], f32, name="bv_sb")
            nc.sync.dma_start(bv_sb[:], bvs[:])
            tri_sb = const.tile([128, 128], bf16, name="tri_sb")
            nc.sync.dma_start(tri_sb[:], tri01[:])
            bo_sb = const.tile([128, E], f32, name="bo_sb")
            nc.sync.dma_start(bo_sb[:], bo_b[:])

            # ---- persistent activations -----------------------------------
            kT = persist.tile([128, NCH, CH], bf16, name="kT")
            qT = persist.tile([128, NCH, CH], bf16, name="qT")
            # V rows per k-token group g; cols 0:64 = h0 feats, 64 = ones,
            # 65:129 = h1 feats, 129 = ones.  AV stationary h = [:, g,
            # 65h:65h+65]; the ones row makes the AV matmul emit softmax
            # denominators in PSUM row 64.
            vh = persist.tile([128, NCH * NSUB, 130], bf16, name="vh")
            nc.vector.tensor_copy(vh[:, :, 64], onesv_sb[:])
            nc.vector.tensor_copy(vh[:, :, 129], onesv_sb[:])

            wo0_sb = persist.tile([128, 4, E], bf16, name="wo0_sb")
            wo1_sb = persist.tile([128, 4, E], bf16, name="wo1_sb")

            cc_in = [dram.tile([NCH, 64, CH], bf16, name=f"cc_in{h}")
                     for h in range(2)]
            cc_out = [dram.tile([NCH, 64, CH], bf16, name=f"cc_out{h}")
                      for h in range(2)]
            warm_in = dram.tile([NCH, 1, 32], bf16, name="warm_in")
            warm_out = dram.tile([NCH, 1, 32], bf16, name="warm_out")
            nc.sync.dma_start(warm_in[:, 0, :], onesv_sb[0:8, 0:32])

            # ---- phase B: QKV projection for one token chunk ---------------
            def emit_b(t):
                if t == 0:
                    xTt = xTt0
                else:
                    xTt = xtp_pool.tile([128, 8, CH], bf16, name="xTt",
                                        tag="xTt")
                    nc.sync.dma_start(xTt[:], xP[:, t])
                for wsb, bias, scale, dest in (
                        (wq_sb, bq_sb, 0.125, qT),
                        (wk_sb, bk_sb, None, kT)):
                    ps = ps_q.tile([128, CH], f32, name="psqk", tag="psq")
                    for e in range(8):
                        nc.tensor.matmul(ps[:], wsb[:, e, :], xTt[:, e, :],
                                         start=(e == 0), stop=(e == 7))
                    if scale is None:
                        nc.vector.tensor_scalar_add(dest[:, t, :], ps[:],
                                                    bias[:])
                    else:
                        nc.vector.tensor_scalar(
                            dest[:, t, :], ps[:], scale, bias[:],
                            ALU.mult, ALU.add)

                psv = ps_q.tile([128, CH], f32, name="psv", tag="psq")
                for e in range(8):
                    nc.tensor.matmul(psv[:], wv_sb[:, e, :], xTt[:, e, :],
                                     start=(e == 0), stop=(e == 7))
                vTs = vstage.tile([128, CH], bf16, name="vTs", tag="vTs")
                nc.vector.tensor_scalar_add(vTs[:], psv[:], bv_sb[:])
                for s in range(NSUB):
                    tv = ps_q.tile([128, 128], bf16, name="tv", tag="psq")
                    nc.tensor.transpose(
                        tv[:], vTs[:, 128 * s:128 * (s + 1)], eye_sb[:])
                    g = NSUB * t + s
                    nc.vector.tensor_copy(vh[:, g, 0:64], tv[:, 0:64])
                    nc.vector.tensor_copy(vh[:, g, 65:129], tv[:, 64:128])

            # ---- phase C: attention for one (chunk, head-half) -------------
            def emit_c(t, h):
                b0 = CPB * (t // CPB)
                pb = 64 * h
                a_ps = ps_a.tile([128, CH], f32, name="a_ps", tag="aps")

                def emit_scores(kc):
                    diag = kc == t
                    pT = ppool.tile([128, NSUB, CH], bf16, name="pT",
                                    tag="pT")
                    for j in range(2):
                        sp = ps_s.tile([128, 2 * CH], f32, name="sp",
                                       tag="sps")
                        for jj in range(2):
                            s = 2 * j + jj
                            q0 = SUB * s if diag else 0
                            nc.tensor.matmul(
                                sp[:, CH * jj + q0:CH * jj + CH],
                                kT[pb:pb + 64, kc, SUB * s:SUB * (s + 1)],
                                qT[pb:pb + 64, t, q0:CH],
                                start=True, stop=True)
                        if diag:
                            for jj in range(2):
                                s = 2 * j + jj
                                q0 = SUB * s
                                nc.scalar.activation(
                                    pT[:, s, q0:CH],
                                    sp[:, CH * jj + q0:CH * jj + CH], AF.Exp)
                                nc.vector.tensor_mul(
                                    pT[:, s, q0:q0 + SUB],
                                    pT[:, s, q0:q0 + SUB], tri_sb[:])
                        else:
                            nc.scalar.activation(
                                pT[:, 2 * j:2 * j + 2, :], sp[:], AF.Exp)
                    return pT

                def emit_av(kc, pT):
                    diag = kc == t
                    for s in range(NSUB):
                        q0 = SUB * s if diag else 0
                        g = NSUB * kc + s
                        nc.tensor.matmul(
                            a_ps[0:65, q0:CH], vh[:, g, 65 * h:65 * h + 65],
                            pT[:, s, q0:CH],
                            start=(kc == b0 and s == 0),
                            stop=(diag and s == NSUB - 1))

                prev = None
                for kc in range(b0, t + 1):
                    pT = emit_scores(kc)
                    if prev is not None:
                        emit_av(*prev)
                    prev = (kc, pT)
                emit_av(*prev)

                den = recpool.tile([1, CH], f32, name="den", tag="den")
                nc.vector.tensor_copy(den[:], a_ps[64:65, :])
                rec = recpool.tile([1, CH], f32, name="rec", tag="rec")
                nc.vector.reciprocal_approx_fast(out=rec[:], in_=den[:])
                bc = bcpool.tile([64, CH], f32, name="bc", tag="bc")
                nc.gpsimd.partition_broadcast(bc[:], rec[:])
                a_sb = apool.tile([64, CH], bf16, name="a_sb", tag="asb")
                nc.vector.tensor_mul(a_sb[:], a_ps[0:64, :], bc[:])
                nc.sync.dma_start(cc_in[h][t, :, :], a_sb[:])

            # ---- main pipeline: QKV(t) | h0-attn(t-1) | h1-attn(t-2) ------
            for t in range(NCH):
                emit_b(t)
                if t == 5:
                    # tiny dummy AllToAll: keeps the CC stream warm so the
                    # real A2A#0 doesn't pay the cold-stream penalty
                    # (~25us vs ~10us observed).  Nothing reads warm_out,
                    # so a slow peer cannot stall local work.
                    nc.gpsimd.collective_compute(
                        "AllToAll", ALU.bypass,
                        ins=[warm_in.opt()], outs=[warm_out.opt()],
                        replica_groups=[list(range(NCORE))])
                if t >= 1:
                    emit_c(t - 1, 0)
                if t >= 2:
                    emit_c(t - 2, 1)
            # wo weights: DMA-idle window once all xP chunks are in flight
            nc.sync.dma_start(wo0_sb[:],
                              woh0.rearrange("(r p) e -> p r e", p=128))
            nc.sync.dma_start(wo1_sb[:],
                              woh1.rearrange("(r p) e -> p r e", p=128))

            emit_c(NCH - 1, 0)
            nc.gpsimd.collective_compute(
                "AllToAll", ALU.bypass,
                ins=[cc_in[0].opt()], outs=[cc_out[0].opt()],
                replica_groups=[list(range(NCORE))])

            aTb = xtp_pool.tile([128, 2, 4, CH], bf16, name="aTb", tag="xTt")
            nc.sync.dma_start(
                aTb[:, 0],
                cc_out[0].rearrange("(a two) f n -> (two f) a n", two=2))

            emit_c(NCH - 2, 1)

            # ---- phase E0: h0 half of the output projection ---------------
            # (deps: A2A#0 + wo0 only — fills PE while h1 tail + A2A#1 run)
            yacc = persist.tile([128, NSUB, E], f32, name="yacc")

            def emit_y0():
                for m in range(NSUB):
                    for nchk in range(2):
                        yps = ps_q.tile([128, CH], f32, name="yps",
                                        tag="psq")
                        for p in range(4):
                            nc.tensor.matmul(
                                yps[:],
                                aTb[:, 0, p, SUB * m:SUB * (m + 1)],
                                wo0_sb[:, p, CH * nchk:CH * (nchk + 1)],
                                start=(p == 0), stop=(p == 3))
                        nc.vector.tensor_add(
                            yacc[:, m, CH * nchk:CH * (nchk + 1)], yps[:],
                            bo_sb[:, CH * nchk:CH * (nchk + 1)])

            emit_c(NCH - 1, 1)
            nc.gpsimd.collective_compute(
                "AllToAll", ALU.bypass,
                ins=[cc_in[1].opt()], outs=[cc_out[1].opt()],
                replica_groups=[list(range(NCORE))])
            emit_y0()
            nc.sync.dma_start(
                aTb[:, 1],
                cc_out[1].rearrange("(a two) f n -> (two f) a n", two=2))

            # ---- phase E1: h1 half + store --------------------------------
            for m in range(NSUB):
                for nchk in range(2):
                    yps = ps_q.tile([128, CH], f32, name="yps", tag="psq")
                    for p in range(4):
                        nc.tensor.matmul(
                            yps[:],
                            aTb[:, 1, p, SUB * m:SUB * (m + 1)],
                            wo1_sb[:, p, CH * nchk:CH * (nchk + 1)],
                            start=(p == 0), stop=(p == 3))
                    ysb = ystage.tile([128, CH], bf16, name="ysb", tag="ysb")
                    nc.vector.tensor_add(
                        ysb[:], yps[:],
                        yacc[:, m, CH * nchk:CH * (nchk + 1)])
                    nc.sync.dma_start(
                        y[SUB * m:SUB * (m + 1),
                          CH * nchk:CH * (nchk + 1)],
                        ysb[:])
    nc.compile()
    return nc


def _prep_in_maps(embd_q, Wq, bq, Wk, bk, Wv, bv, Wo, bo):
    import ml_dtypes
    bf16 = ml_dtypes.bfloat16
    x = embd_q.reshape(NTOK, E).astype(np.float32)
    # xP[p, t, e, n] = x[t*512+n, e*128+p]: 8KB contiguous per partition
    # per chunk
    xPm = np.ascontiguousarray(
        x.reshape(NCH, CH, 8, 128).transpose(3, 0, 2, 1).astype(bf16))
    eye = np.eye(128, dtype=bf16)
    r = np.arange(128)
    # pT is [k-part, q-col]; mask out k > q (future tokens)
    tri01 = np.ascontiguousarray(
        np.where(r[:, None] > r[None, :], 0.0, 1.0).astype(bf16))
    ones_v = np.ones((128, NCH * NSUB), dtype=bf16)
    bo_b = np.ascontiguousarray(
        np.broadcast_to(bo.astype(np.float32), (128, E)))
    woTf = Wo.astype(np.float32).T  # [feat, out]
    # pair-interleaved per-head layouts: partition q of pair p maps to
    # feat = 128*(2p) + q  (q < 64, even kt)  or  128*(2p+1) + (q-64)
    idx = np.zeros((4, 128), dtype=np.int64)
    for p in range(4):
        idx[p, :64] = 128 * (2 * p) + np.arange(64)
        idx[p, 64:] = 128 * (2 * p + 1) + np.arange(64)
    woh0 = np.ascontiguousarray(woTf[idx.reshape(-1)].astype(bf16))
    woh1 = np.ascontiguousarray(woTf[(idx + 64).reshape(-1)].astype(bf16))

    def wlayout(W, sl):
        # [E, 128] -> [p, e, m]: contiguous 2KB/partition DMA segments
        wT = W[sl].astype(np.float32).T.astype(bf16)
        return np.ascontiguousarray(
            wT.reshape(8, 128, 128).transpose(1, 0, 2))

    in_maps = []
    for c in range(NCORE):
        sl = slice(128 * c, 128 * (c + 1))
        in_maps.append({
            "xP": xPm,
            "wqT": wlayout(Wq, sl),
            "wkT": wlayout(Wk, sl),
            "wvT": wlayout(Wv, sl),
            "woh0": woh0,
            "woh1": woh1,
            "bqs": np.ascontiguousarray(
                (bq[sl] * 0.125).reshape(128, 1), dtype=np.float32),
            "bks": np.ascontiguousarray(bk[sl].reshape(128, 1),
                                        dtype=np.float32),
            "bvs": np.ascontiguousarray(bv[sl].reshape(128, 1),
                                        dtype=np.float32),
            "bo_b": bo_b,
            "eye": eye,
            "tri01": tri01,
            "ones_v": ones_v,
        })
    return in_maps


def kernel(embd_q, Wq, bq, Wk, bk, Wv, bv, Wo, bo, _trace=False):
    if "nc" not in _nc_cache:
        _nc_cache["nc"] = build_nc()
    in_maps = _prep_in_maps(np.asarray(embd_q), np.asarray(Wq), np.asarray(bq),
                            np.asarray(Wk), np.asarray(bk), np.asarray(Wv),
                            np.asarray(bv), np.asarray(Wo), np.asarray(bo))
    import os
    tc_env = os.environ.get("TRACE_CORES")
    res = run_bass_kernel_spmd(
        _nc_cache["nc"], in_maps, list(range(NCORE)), trace=_trace,
        trace_cores=(list(range(NCORE)) if tc_env else None))
    out = np.concatenate(
        [np.asarray(res.results[c]["y"]).astype(np.float32)
         for c in range(NCORE)], axis=0)
    out = out.reshape(B, T, E)
    kernel.last_results = res
    return out
